# revision 80
# baseline (speedup 1.0000x reference)
"""Trainium2 Bass kernel for nn_EventPairCompositionModel (fp8 DoubleRow).

Strategy (data-parallel over batch, 8 cores, B=512 -> 64 per core):
  - Host builds a per-core compact table of QUAD rows: the ~8.2K unique
    (idx0..idx3) context/event tuples each shard touches, stored as fp8e4m3
    (x8 scale) 1200-elem rows padded to 1280 bytes and DECLARED int16 so the
    SWDGE transpose-gather's 16-bit granularity lands fp8 element pairs
    (2u, 2u+1) on partition u%128 -- exactly the [K, 2, N] layout
    MatmulPerfMode.DoubleRow wants.  Quad rows give the minimal 5 DoubleRow
    K-pairs per m-tile and 1 gather index per (b, n) (the SWDGE ucode is
    per-index bound; bursts over ~512 idx deadlock its descriptor ring).
  - Context MLP (1200->512->256) entirely in fp8 DoubleRow (2 K-rows/cycle).
    Weight passes of 3 groups of 512 (b,n) pairs amortize each stationary
    load over 3 matmuls while leaving PSUM for a 3-deep MLP2 ring (no
    act-wait bubble at pass boundaries); redundant LDWEIGHTS are removed by
    a post-compile pass.  The 64 event tuples ride pass 0 as a narrow group.
  - Cosine numerators/|c|^2 via per-batch fp8 DR matmuls ([p,2,1]
    stationaries) deferred into the NEXT pass's matmul stream, staged to
    [1, 8192] rows and reshaped to [64 batch, 128 ctx] by one DMA; batched
    KNRM pooling in two column chunks x two batch halves (first half
    mid-stream).  All scalar activations live in one function table
    (sqrt/sigmoid rewritten via exp/ln) so no mid-kernel table reloads.
  - Distance-kernel path in bf16 off the event gather; final linear+sigmoid
    in [64 batch partitions, feature] layout.
All 8 cores run the identical program on their own batch shard (SPMD).
Host falls back to a numpy reference for pathological inputs (out-of-range
indices); the quad table always fits for well-formed ones.
"""

import numpy as np
import ml_dtypes

import concourse.bacc as bacc
import concourse.bass as bass
import concourse.tile as tile
import concourse.mybir as mybir
from concourse.bass_utils import run_bass_kernel_spmd
from concourse import library_config

F32 = mybir.dt.float32
BF16 = mybir.dt.bfloat16
FP8 = mybir.dt.float8e4
I16 = mybir.dt.int16
AF = mybir.ActivationFunctionType
DR = mybir.MatmulPerfMode.DoubleRow

# Problem shapes (hardcoded per spec)
B, N, C, E = 512, 128, 4, 300
V = 50000
H1, H2 = 512, 256
NF, NK = 8, 11
NCORES = 8
BC = B // NCORES          # 64 batches per core
CT = 16384                # compact quad-table rows (int16-indexable)
EP = 1200                 # elems per quad row (all 4 components)
RU = 640                  # int16 units per table row (1280 fp8 elems)
GROUPS = (BC * N) // 512  # 16 groups of 512 (b,n) pairs
NKP = 5                   # MLP1 K-pairs per m-tile (5 slots of 256 elems)
FD = 28                   # features: 9 dist + 8 extracted + 11 kp
# weight passes: 3 groups share each stationary load; PSUM then affords a
# 3-deep MLP2 ring (no act-wait bubble at pass boundaries)
PASS_GROUPS = [[0, 1, 2], [3, 4, 5], [6, 7, 8], [9, 10, 11], [12, 13], [14, 15]]

MUS = [1.0, 0.9, 0.7, 0.5, 0.3, 0.1, -0.1, -0.3, -0.5, -0.7, -0.9]
SIGMAS = [1e-3] + [0.1] * 10

_PROGRAM_CACHE = {}


def _build_program(fast: bool = True):
    if True in _PROGRAM_CACHE:
        return _PROGRAM_CACHE[True]

    # Force the activation-table placement pass to use the combined
    # natural_log_exp_and_others set (covers relu/exp/ln/square/copy): blank
    # the narrower sets, keeping list positions so act_func_set_ids stay
    # aligned with act_info.json.  Every function this kernel uses then lives
    # in one table -> no 1.3us ACT_TABLE_LOADs between activations.
    from concourse import hw_specs as _hw
    if not hasattr(bacc, "_orig_gat"):
        bacc._orig_gat = bacc.get_activation_tables

        def _gat(arch):
            t = dict(bacc._orig_gat(arch))
            keys = list(t.keys())
            for i, k in enumerate(keys):
                if k != "natural_log_exp_and_others" and i < 6:
                    t[k] = set()
            return t

        bacc.get_activation_tables = _gat

    nc = bacc.Bacc("TRN2", target_bir_lowering=False, debug=False, num_swdge_queues=4)

    # ---- DRAM I/O ----
    ctab = nc.dram_tensor("ctab", (CT, RU), I16, kind="ExternalInput")
    cidx = nc.dram_tensor("cidx", (128, GROUPS * 2 * 16), I16, kind="ExternalInput")
    eidx = nc.dram_tensor("eidx", (128, 8), I16, kind="ExternalInput")
    w1p = nc.dram_tensor("w1p", (128, NKP * 4 * 256), FP8, kind="ExternalInput")
    w2p = nc.dram_tensor("w2p", (128, 2 * 2 * 256), FP8, kind="ExternalInput")
    wvp = nc.dram_tensor("wvp", (128, 4 * 9), BF16, kind="ExternalInput")
    b1d = nc.dram_tensor("b1d", (128, 4), F32, kind="ExternalInput")
    b1f = nc.dram_tensor("b1f", (128, 4), F32, kind="ExternalInput")
    b2d = nc.dram_tensor("b2d", (128, 2), F32, kind="ExternalInput")
    ebv = nc.dram_tensor("ebv", (BC, 9), F32, kind="ExternalInput")
    ndsq = nc.dram_tensor("ndsq", (BC, 9), F32, kind="ExternalInput")
    featd = nc.dram_tensor("featd", (BC, NF), F32, kind="ExternalInput")
    wcr = nc.dram_tensor("wcr", (BC, FD), F32, kind="ExternalInput")
    bcr = nc.dram_tensor("bcr", (BC, 1), F32, kind="ExternalInput")
    out_d = nc.dram_tensor("out", (BC, 1), F32, kind="ExternalOutput")

    with tile.TileContext(nc) as tc:
        with (
            tc.tile_pool(name="consts", bufs=1) as cpool,
            tc.tile_pool(name="xt", bufs=9) as xtpool,
            tc.tile_pool(name="s1", bufs=6) as s1pool,
            tc.tile_pool(name="s2", bufs=8) as s2pool,
            tc.tile_pool(name="csq", bufs=8) as csqpool,
            tc.tile_pool(name="small", bufs=2) as smpool,
            tc.tile_pool(name="pm1", bufs=3, space="PSUM") as pm1,
            tc.tile_pool(name="pm2", bufs=3, space="PSUM") as pm2,
            tc.tile_pool(name="pg", bufs=2, space="PSUM") as pgpool,
        ):
            nc.gpsimd.load_library(library_config.mlp)

            # ---- load constants ----
            cidx_s = cpool.tile([128, GROUPS * 2 * 16], I16)
            nc.sync.dma_start(cidx_s[:], cidx.ap())
            eidx_s = cpool.tile([128, 8], I16)
            nc.sync.dma_start(eidx_s[:], eidx.ap())
            w1p_s = cpool.tile([128, NKP * 4 * 256], FP8)
            nc.sync.dma_start(w1p_s[:], w1p.ap())
            w2p_s = cpool.tile([128, 2 * 2 * 256], FP8)
            nc.scalar.dma_start(w2p_s[:], w2p.ap())
            wvp_s = cpool.tile([128, 4 * 9], BF16)
            nc.scalar.dma_start(wvp_s[:], wvp.ap())
            b1_s = cpool.tile([128, 4], F32)
            nc.sync.dma_start(b1_s[:], b1d.ap())
            b1f_s = cpool.tile([128, 4], F32)
            nc.sync.dma_start(b1f_s[:], b1f.ap())
            b2_s = cpool.tile([128, 2], F32)
            nc.sync.dma_start(b2_s[:], b2d.ap())
            ebv_s = cpool.tile([BC, 9], F32)
            nc.scalar.dma_start(ebv_s[:], ebv.ap())
            ndsq_s = cpool.tile([BC, 9], F32)
            nc.scalar.dma_start(ndsq_s[:], ndsq.ap())
            wcr_s = cpool.tile([BC, FD], F32)
            nc.scalar.dma_start(wcr_s[:], wcr.ap())
            bcr_s = cpool.tile([BC, 1], F32)
            nc.scalar.dma_start(bcr_s[:], bcr.ap())
            F_s = cpool.tile([BC, FD], F32)
            nc.sync.dma_start(F_s[:, 9 : 9 + NF], featd.ap())

            # ones pair for |c|^2 matmuls; K-group dim at stride 16 to satisfy
            # the dual-fp8 LDWEIGHTS AP restriction (pair step % 16 == 0)
            ones2c_s = cpool.tile([128, 32], FP8)
            nc.vector.memset(ones2c_s[:], 1.0)
            # batched KNRM constants: [64, k(11), 128] of -mu_k / -1/(2 sig_k^2)
            mub_s = cpool.tile([BC, NK * 128], F32)
            i2s_s = cpool.tile([BC, NK * 128], F32)
            for k in range(NK):
                nc.vector.memset(mub_s[:, 128 * k : 128 * (k + 1)], -MUS[k])
                nc.vector.memset(
                    i2s_s[:, 128 * k : 128 * (k + 1)],
                    -1.0 / (2.0 * SIGMAS[k] ** 2),
                )

            # persistent accumulators
            traw_s = cpool.tile([BC, 128], F32)        # 16*dot per (b, n)
            drow_s = cpool.tile([1, 512 * GROUPS], F32)  # dots, (g,s,n) on part 0
            nrow_s = cpool.tile([1, 512 * GROUPS], F32)  # |c|^2 likewise
            sgram_s = cpool.tile([128, 2 * BC], FP8)   # event reprs [p, m(2), 64]
            cse_s = cpool.tile([128, 2 * BC], FP8)     # their squares
            s1e_s = cpool.tile([128, 4 * BC], FP8)     # event s1 [p, mj(4), 64]
            predb_s = cpool.tile([128, 4 * BC], BF16)  # predicates [p, f(4), 64]
            ne2_s = cpool.tile([BC, 1], F32)
            ncsq0_s = cpool.tile([BC, 128], F32)
            trans_s = cpool.tile([BC, 128], F32)
            pooled_s = cpool.tile([BC, NK], F32)

            # ---- gathers (issued lazily so pool-slot reuse stays WAR-safe) ----
            xe_s = cpool.tile([128, 5 * 128], I16)   # event quad gather
            xts = {}

            def issue_gather(g):
                # two 256-idx gathers per group (small bursts keep the SWDGE
                # descriptor-ring carveout happy; big ones deadlock it)
                if g >= GROUPS:
                    return
                xt = xtpool.tile([128, 5 * 512], I16, tag="xt", name=f"xt{g}")
                nc.gpsimd.dma_gather(
                    out_ap=xt[:].rearrange("p (s r) -> p s r", s=5),
                    in_ap=ctab.ap(),
                    idxs_ap=cidx_s[:, 32 * g : 32 * (g + 1)],
                    num_idxs=512,
                    num_idxs_reg=512,
                    elem_size=RU,
                    transpose=True,
                )
                xts[g] = xt

            # g0 first (it gates the first matmul); events slot in behind it
            issue_gather(0)
            nc.gpsimd.dma_gather(
                out_ap=xe_s[:].rearrange("p (s i) -> p s i", s=5),
                in_ap=ctab.ap(),
                idxs_ap=eidx_s[:],
                num_idxs=128,
                num_idxs_reg=128,
                elem_size=RU,
                transpose=True,
            )
            for g in PASS_GROUPS[0][1:] + PASS_GROUPS[1]:
                issue_gather(g)

            def ctx_rhs(g, uj, w):
                # [p, 2(byte), w cols] fp8 view: group g, K-slot uj
                v = xts[g][:].bitcast(FP8).rearrange(
                    "p (s r i) -> p s i r", s=5, i=2
                )
                return v[:, uj, :, 0:w]

            def evt_rhs(uj):
                v = xe_s[:].bitcast(FP8).rearrange("p (s r i) -> p s i r", s=5, i=2)
                return v[:, uj, :, 0:BC]

            def w1_ap(kp, m):
                return w1p_s[:].rearrange(
                    "p (kp m i c) -> p kp m i c", kp=NKP, m=4, i=2
                )[:, kp, m, :, :]

            def w2_ap(q, m):
                return w2p_s[:].rearrange(
                    "p (q m i c) -> p q m i c", q=2, m=2, i=2
                )[:, q, m, :, :]

            # deferred per-batch dot/|c|^2 matmuls: flushed into the NEXT
            # pass's matmul stream so their dependency chains (act -> square)
            # never stall the PE at pass boundaries.  Outputs pack 4 streams
            # per PSUM bank at 32-aligned partitions.
            pending = []

            flush_n = [0]

            def flush_step(nstreams=2):
                # emit up to nstreams deferred dot/norm streams; called once
                # per m-slot of the following pass so the ring-2 PSUM recycle
                # always has a full m-window of slack
                sg_v = sgram_s[:].rearrange("p (m c) -> p m c", m=2)
                on_v = ones2c_s[:].rearrange("p (i x) -> p i x", i=2)[:, :, 0:1]
                for _ in range(nstreams):
                    if not pending:
                        return
                    kind, g, mv = pending.pop(0)
                    flush_n[0] += 1
                    PD = pgpool.tile([1, 512], F32, tag="pg", name=f"pd{flush_n[0]}")
                    mv_v = mv[:].rearrange("p (m x) -> p m x", m=2)
                    for s in range(4):
                        lane = 4 * g + s
                        nc.tensor.matmul(
                            PD[:, 128 * s : 128 * (s + 1)],
                            sg_v[:, :, lane : lane + 1] if kind == "d" else on_v,
                            mv_v[:, :, 128 * s : 128 * (s + 1)],
                            start=True, stop=True, perf_mode=DR,
                        )
                    if kind == "d":
                        nc.scalar.copy(drow_s[:, 512 * g : 512 * (g + 1)], PD[:])
                    else:
                        nc.vector.tensor_copy(
                            out=nrow_s[:, 512 * g : 512 * (g + 1)], in_=PD[:]
                        )

            def tail_half(h):
                # cosine + batched KNRM pooling for batches 32h..32h+31
                r = slice(32 * h, 32 * (h + 1))
                # scalar-issued (the sync queue lags far behind on semaphore
                # bookkeeping); ncsq first -- the norm-product chain consumes
                # it before traw is needed
                nc.scalar.dma_start(
                    ncsq0_s[r, :], nrow_s[:, 4096 * h : 4096 * (h + 1)]
                )
                nc.scalar.dma_start(
                    traw_s[r, :], drow_s[:, 4096 * h : 4096 * (h + 1)]
                )
                prodn = smpool.tile([BC, 128], F32, tag="smT", name=f"prodn{h}")
                nc.vector.tensor_tensor(
                    out=prodn[r, :], in0=ncsq0_s[r, :],
                    in1=ne2_s[r, :].broadcast_to([32, 128]),
                    op=mybir.AluOpType.mult,
                )
                # 1/sqrt(x) = exp(-0.5 ln x): stays inside the ln+exp
                # activation-table set (table reloads cost 1.3us each)
                lnp = smpool.tile([BC, 128], F32, tag="smT", name=f"lnp{h}")
                nc.scalar.activation(lnp[r, :], prodn[r, :], AF.Ln)
                nf = smpool.tile([BC, 128], F32, tag="smT", name=f"nf{h}")
                nc.scalar.activation(nf[r, :], lnp[r, :], AF.Exp, scale=-0.5)
                nc.vector.tensor_mul(trans_s[r, :], traw_s[r, :], nf[r, :])

                # two kernel-chunks so the scalar Exp of chunk A overlaps the
                # DVE chain of chunk B
                ekbs = []
                for k0, k1 in ((0, 6), (6, NK)):
                    c = slice(128 * k0, 128 * k1)
                    nk = k1 - k0
                    dk = smpool.tile(
                        [BC, NK * 128], F32, tag="smB", name=f"dk{h}{k0}"
                    )
                    nc.vector.tensor_tensor(
                        out=dk[r, c],
                        in0=trans_s[r, :][:, None, :].broadcast_to([32, nk, 128]),
                        in1=mub_s[r, c].rearrange("b (k n) -> b k n", k=nk),
                        op=mybir.AluOpType.add,
                    )
                    dsq = smpool.tile(
                        [BC, NK * 128], F32, tag="smB", name=f"dsq{h}{k0}"
                    )
                    nc.vector.tensor_mul(dsq[r, c], dk[r, c], dk[r, c])
                    argb = smpool.tile(
                        [BC, NK * 128], F32, tag="smB", name=f"argb{h}{k0}"
                    )
                    nc.vector.tensor_mul(argb[r, c], dsq[r, c], i2s_s[r, c])
                    argc = smpool.tile(
                        [BC, NK * 128], F32, tag="smB", name=f"argc{h}{k0}"
                    )
                    nc.vector.tensor_scalar_max(argc[r, c], argb[r, c], -87.0)
                    ekb = smpool.tile(
                        [BC, NK * 128], F32, tag=f"smE{k0}", name=f"ekb{h}{k0}"
                    )
                    nc.scalar.activation(ekb[r, c], argc[r, c], AF.Exp)
                    ekbs.append((k0, k1, ekb))
                for k0, k1, ekb in ekbs:
                    c = slice(128 * k0, 128 * k1)
                    nc.vector.reduce_sum(
                        out=pooled_s[r, k0:k1],
                        in_=ekb[r, c].rearrange("b (k n) -> b k n", k=k1 - k0),
                        axis=mybir.AxisListType.X,
                    )

            # ---- weight passes (+ events on pass 0) ----
            for pi, grp in enumerate(PASS_GROUPS):
                with_evt = pi == 0
                # MLP1
                s1t = {}
                for g in grp:
                    s1t[g] = s1pool.tile([128, 4 * 512], FP8, tag="s1", name=f"s1_{g}")
                pl = {}
                for m in range(4):
                    for g in grp:
                        pl[g] = pm1.tile([128, 512], F32, tag="pm1", name=f"p1_{g}_{m}")
                    if with_evt:
                        pE = pm2.tile([128, BC], F32, tag="pm2", name=f"pe_{m}")
                    for kp in range(NKP):
                        st, sp = kp == 0, kp == NKP - 1
                        for g in grp:
                            nc.tensor.matmul(
                                pl[g][:], w1_ap(kp, m), ctx_rhs(g, kp, 512),
                                start=st, stop=sp, perf_mode=DR,
                            )
                        if with_evt:
                            nc.tensor.matmul(
                                pE[:], w1_ap(kp, m), evt_rhs(kp),
                                start=st, stop=sp, perf_mode=DR,
                            )
                    # acts first (PSUM recycle gates the next m-slot's
                    # matmuls), then the deferred dots ride behind them
                    for g in grp:
                        nc.scalar.activation(
                            s1t[g][:, 512 * m : 512 * (m + 1)], pl[g][:],
                            AF.Relu, bias=b1_s[:, m : m + 1], scale=1.0 / 16,
                        )
                    flush_step(2)
                    if with_evt:
                        nc.scalar.activation(
                            s1e_s[:, BC * m : BC * (m + 1)], pE[:],
                            AF.Relu, bias=b1_s[:, m : m + 1], scale=1.0 / 16,
                        )

                # MLP2 in half-passes of <=2 groups (ring-3 PSUM slack)
                s2t = {}
                for g in grp:
                    s2t[g] = s2pool.tile([128, 2 * 512], FP8, tag="s2", name=f"s2_{g}")
                for half in range((len(grp) + 1) // 2):
                    gh = grp[2 * half : 2 * half + 2]
                    for m in range(2):
                        p2 = {}
                        for g in gh:
                            p2[g] = pm2.tile(
                                [128, 512], F32, tag="pm2", name=f"p2_{g}_{m}"
                            )
                        if with_evt and half == 0:
                            pE2 = pm2.tile([128, BC], F32, tag="pm2", name=f"pe2_{m}")
                        for q in range(2):
                            st, sp = q == 0, q == 1
                            for g in gh:
                                nc.tensor.matmul(
                                    p2[g][:], w2_ap(q, m),
                                    s1t[g][:].rearrange("p (mj x) -> p mj x", mj=4)[
                                        :, 2 * q : 2 * q + 2, :
                                    ],
                                    start=st, stop=sp, perf_mode=DR,
                                )
                            if with_evt and half == 0:
                                nc.tensor.matmul(
                                    pE2[:], w2_ap(q, m),
                                    s1e_s[:].rearrange("p (mj x) -> p mj x", mj=4)[
                                        :, 2 * q : 2 * q + 2, :
                                    ],
                                    start=st, stop=sp, perf_mode=DR,
                                )
                        for g in gh:
                            nc.scalar.activation(
                                s2t[g][:, 512 * m : 512 * (m + 1)], p2[g][:],
                                AF.Relu, bias=b2_s[:, m : m + 1], scale=1.0 / 8,
                            )
                        if with_evt and half == 0:
                            nc.scalar.activation(
                                sgram_s[:, BC * m : BC * (m + 1)], pE2[:],
                                AF.Relu, bias=b2_s[:, m : m + 1], scale=1.0 / 8,
                            )

                    if with_evt and half == 0:
                        # event extras: squares, |e|^2, predicates, variances
                        nc.vector.tensor_mul(cse_s[:], sgram_s[:], sgram_s[:])
                        pne = pm2.tile([BC, 1], F32, tag="pm2", name="pne")
                        nc.tensor.matmul(
                            pne[:],
                            cse_s[:].rearrange("p (m c) -> p m c", m=2),
                            ones2c_s[:, 0:2].rearrange("p (o i) -> p i o", i=2),
                            start=True, stop=True, perf_mode=DR,
                        )
                        nc.scalar.copy(ne2_s[:], pne[:])

                        # predicates: quad-row elems 300..599 (component 1)
                        nc.scalar.copy(
                            predb_s[:].rearrange("p (s i l) -> p s i l", s=2, i=2),
                            xe_s[:].bitcast(FP8).rearrange(
                                "p (s r i) -> p s i r", s=5, i=2
                            )[:, 1:3, :, 0:BC],
                        )
                        pvar = pm2.tile([BC, 9], F32, tag="pm2", name="pvar")
                        for f in range(4):
                            nc.tensor.matmul(
                                pvar[:],
                                predb_s[:].rearrange("p (f l) -> p f l", f=4)[:, f, :],
                                wvp_s[:].rearrange("p (f v) -> p f v", f=4)[:, f, :],
                                start=(f == 0), stop=(f == 3),
                            )
                        ez = smpool.tile([BC, 9], F32, tag="sm9", name="ez")
                        nc.scalar.activation(ez[:], pvar[:], AF.Exp, scale=1.0 / 8)
                        ezb = smpool.tile([BC, 9], F32, tag="sm9", name="ezb")
                        nc.vector.tensor_mul(ezb[:], ez[:], ebv_s[:])
                        ez1 = smpool.tile([BC, 9], F32, tag="sm9", name="ez1")
                        nc.vector.tensor_scalar_add(ez1[:], ezb[:], 1.0)
                        var = smpool.tile([BC, 9], F32, tag="sm9", name="var")
                        nc.scalar.activation(var[:], ez1[:], AF.Ln)
                        rv = smpool.tile([BC, 9], F32, tag="sm9", name="rv")
                        nc.vector.reciprocal(rv[:], var[:])
                        qd = smpool.tile([BC, 9], F32, tag="sm9", name="qd")
                        nc.vector.tensor_mul(qd[:], ndsq_s[:], rv[:])
                        nc.scalar.activation(F_s[:, 0:9], qd[:], AF.Exp)

                    # squared activations now; dot/norm matmuls deferred
                    for g in gh:
                        csq = csqpool.tile(
                            [128, 2 * 512], FP8, tag="csq", name=f"csq_{g}"
                        )
                        nc.vector.tensor_mul(csq[:], s2t[g][:], s2t[g][:])
                        pending.append(("d", g, s2t[g]))
                        pending.append(("n", g, csq))
                # prefetch the gathers needed two passes ahead
                if pi + 2 < len(PASS_GROUPS):
                    for g in PASS_GROUPS[pi + 2]:
                        issue_gather(g)
                if pi == 3:
                    tail_half(0)   # batches 0..31 finished flushing by now
            while pending:
                flush_step(2)
            tail_half(1)

            # ---- final score ----
            poolc = smpool.tile([BC, NK], F32, tag="smK", name="poolc")
            nc.vector.tensor_scalar_max(poolc[:], pooled_s[:], 1e-10)
            nc.scalar.activation(F_s[:, 9 + NF :], poolc[:], AF.Ln)

            fw = smpool.tile([BC, FD], F32, tag="smK", name="fw")
            nc.vector.tensor_mul(fw[:], F_s[:], wcr_s[:])
            sc = smpool.tile([BC, 1], F32, tag="smS", name="sc")
            nc.vector.reduce_sum(out=sc[:], in_=fw[:], axis=mybir.AxisListType.X)
            # sigmoid via exp (avoids a sigmoid-table load): 1/(1+e^(-x-bc))
            emx = smpool.tile([BC, 1], F32, tag="smS", name="emx")
            nc.scalar.activation(emx[:], sc[:], AF.Exp, bias=bcr_s[:], scale=-1.0)
            em1 = smpool.tile([BC, 1], F32, tag="smS", name="em1")
            nc.vector.tensor_scalar_add(em1[:], emx[:], 1.0)
            sig = smpool.tile([BC, 1], F32, tag="smS", name="sig")
            nc.vector.reciprocal(sig[:], em1[:])
            nc.scalar.dma_start(out_d.ap(), sig[:])

    nc.compile()

    # Spread SWDGE gathers across the 4 queues (ucode locks each DMASW
    # semaphore lane to one queue; lanes are assigned round-robin in
    # scheduled order).
    import re as _re
    for blk in nc.m.functions[0].blocks:
        for inst in blk.instructions:
            if type(inst).__name__ == "InstDMAGatherAnt":
                for u in inst.sync_info.on_update:
                    m = _re.match(r"DMASW(\d+)_", u.ant_name or "")
                    if m:
                        inst.queue_num = int(m.group(1)) % 4
                        break

    _dedup_ldweights(nc)

    _PROGRAM_CACHE[True] = nc
    return nc


def _ldw_sig(inst):
    a = inst.ins[0]
    return (
        a.memref,
        a.offset,
        tuple(tuple(d) for d in a.ap),
        getattr(inst, "perf_mode", None),
        getattr(inst, "tile_position", None),
        getattr(inst, "tile_size", None),
        getattr(inst, "is_transpose", None),
    )


def _dedup_ldweights(nc):
    """Remove InstLdweights that reload the stationary operand already in the
    PE array.  The compile pass splits every matmul into LDWEIGHTS+MATMUL;
    back-to-back matmuls sharing weights then pay a redundant ~200ns load.
    Conservative: only drops loads carrying no semaphore waits/updates, so
    cross-engine ordering is untouched."""
    dropped = 0
    for blk in nc.m.functions[0].blocks:
        cur = None          # signature currently in the array
        keep = []
        for inst in blk.instructions:
            nm = type(inst).__name__
            if nm == "InstLdweights":
                sig = _ldw_sig(inst)
                si = inst.sync_info
                if sig == cur and (
                    si is None or (not si.on_wait and not si.on_update)
                ):
                    dropped += 1
                    continue
                cur = sig
            keep.append(inst)
        blk.instructions = keep
    return dropped


def _wrap16(flat_idx):
    """int16 index list -> (128, n/16) tile layout replicated into 8 stripes."""
    n = flat_idx.shape[0]
    t = np.zeros((16, n // 16), np.int16)
    t[np.arange(n) % 16, np.arange(n) // 16] = flat_idx
    return np.tile(t, (8, 1))


FP8NP = ml_dtypes.float8_e4m3fn


def _prep_core_inputs(inputs, core, fast=True, table8=None):
    """Host-side shard + weight re-layouts for one core."""
    W1 = np.asarray(inputs["W1"], np.float32)
    W2 = np.asarray(inputs["W2"], np.float32)
    Wv = np.asarray(inputs["Wv"], np.float32)
    Wc = np.asarray(inputs["Wc"], np.float32)
    b1 = np.asarray(inputs["b1"], np.float32)
    b2 = np.asarray(inputs["b2"], np.float32)
    bv = np.asarray(inputs["bv"], np.float32)
    bc = np.asarray(inputs["bc"], np.float32)

    sl = slice(core * BC, (core + 1) * BC)
    ev = np.asarray(inputs["batch_event"][sl], np.int64)          # (BC, C)
    feats = np.asarray(inputs["batch_features"][sl], np.float32)  # (BC, NF)
    dists = np.asarray(inputs["batch_distances"][sl], np.float32) # (BC, 9)
    ctx = np.asarray(inputs["batch_context"][sl], np.int64)       # (BC, N, C)

    if table8 is None:
        table8 = (np.asarray(inputs["event_table"], np.float32) * 8.0).astype(FP8NP)

    # quad keys: the full (idx0..idx3) tuple per (b, n) / event
    Vp = np.int64(V + 1)
    ctxq = ctx.reshape(BC * N, 4)
    evq = ev.reshape(BC, 4)

    def qkey(a):
        return ((a[:, 0] * Vp + a[:, 1]) * Vp + a[:, 2]) * Vp + a[:, 3]

    keys = np.concatenate([qkey(ctxq), qkey(evq)])
    uniq, inv = np.unique(keys, return_inverse=True)
    assert len(uniq) <= CT
    ctab8 = np.zeros((CT, 2 * RU), FP8NP)
    rem = uniq.copy()
    for c in range(3, -1, -1):
        ctab8[: len(uniq), E * c : E * (c + 1)] = table8[rem % Vp]
        rem //= Vp
    rctx = inv[: BC * N].astype(np.int16).reshape(BC, N)
    rev = inv[BC * N :].astype(np.int16)

    # context gathers: per group g, 512 idxs ordered (s, n)
    ci = rctx.reshape(GROUPS, 512)
    cidx = np.concatenate(
        [_wrap16(ci[g]) for g in range(GROUPS)], axis=1
    )
    # event gather: 128 idxs; lanes >= BC gather row 0
    ei = np.zeros(128, np.int16)
    ei[:BC] = rev

    # W1 packed for DoubleRow: [p, kp(uj), m, i, mcol]
    W1x = (8.0 * W1).astype(np.float32)          # (H1, C*E)
    W2x = (8.0 * W2).astype(np.float32)          # (H2, H1)
    p_i = np.arange(128)
    w1p = np.zeros((128, NKP, 4, 2, 128), np.float32)
    for uj in range(NKP):
        e = 256 * uj + 2 * p_i[:, None] + np.arange(2)[None, :]  # (128, 2)
        valid = e < EP
        src = W1x[:, np.minimum(e, EP - 1)] * valid[None, :, :]   # (H1, 128, 2)
        blk = src.reshape(4, 128, 128, 2).transpose(2, 0, 3, 1)
        w1p[:, uj] = blk
    w2p = np.zeros((128, 2, 2, 2, 128), np.float32)
    for q in range(2):
        for i in range(2):
            src = W2x[:, 128 * (2 * q + i) + p_i]      # (H2, 128)
            w2p[:, q, :, i, :] = src.reshape(2, 128, 128).transpose(2, 0, 1)
    # predicates live at quad-row elems 300..599 (component 1):
    # f slots are (uj, i) for uj in {1, 2}
    wvp = np.zeros((128, 4, 9), np.float32)
    for f in range(4):
        e = 256 * (1 + f // 2) + 2 * p_i + (f % 2)
        k = e - E
        valid = (k >= 0) & (k < E)
        wvp[:, f, :] = Wv[:, np.clip(k, 0, E - 1)].T * valid[:, None]

    wc_r = np.concatenate(
        [Wc[0, 0:9], Wc[0, 9 : 9 + NF], Wc[0, 9 + NF :] * 0.01]
    ).astype(np.float32)

    m = {
        "ctab": np.ascontiguousarray(ctab8).view(np.int16),
        "cidx": np.ascontiguousarray(cidx),
        "eidx": np.ascontiguousarray(_wrap16(ei)),
        "w1p": w1p.reshape(128, -1).astype(FP8NP),
        "w2p": w2p.reshape(128, -1).astype(FP8NP),
        "wvp": wvp.reshape(128, -1).astype(ml_dtypes.bfloat16),
        "b1d": np.ascontiguousarray(4.0 * b1.reshape(4, 128).T),
        "b1f": np.ascontiguousarray(64.0 * b1.reshape(4, 128).T),
        "b2d": np.ascontiguousarray(4.0 * b2.reshape(2, 128).T),
        "ebv": np.tile(np.exp(bv)[None, :], (BC, 1)).astype(np.float32),
        "ndsq": np.ascontiguousarray(-(dists * dists)),
        "featd": np.ascontiguousarray(feats),
        "wcr": np.tile(wc_r[None, :], (BC, 1)),
        "bcr": np.full((BC, 1), -bc[0], np.float32),
    }
    return m


def _numpy_reference(inputs):
    """Pure-host fallback (unreachable for the spec's random fill)."""
    t = np.asarray(inputs["event_table"], np.float32)
    W1 = np.asarray(inputs["W1"], np.float32); b1 = np.asarray(inputs["b1"], np.float32)
    W2 = np.asarray(inputs["W2"], np.float32); b2 = np.asarray(inputs["b2"], np.float32)
    Wv = np.asarray(inputs["Wv"], np.float32); bv = np.asarray(inputs["bv"], np.float32)
    Wc = np.asarray(inputs["Wc"], np.float32); bc = np.asarray(inputs["bc"], np.float32)
    be = np.asarray(inputs["batch_event"], np.int64)
    bf = np.asarray(inputs["batch_features"], np.float32)
    bd = np.asarray(inputs["batch_distances"], np.float32)
    bx = np.asarray(inputs["batch_context"], np.int64)
    ee = t[be]                                    # (B, C, E)
    ce = t[bx]                                    # (B, N, C, E)
    pred = ee[:, 1, :]
    zv = pred @ Wv.T + bv
    var = np.log1p(np.exp(zv))
    de = np.exp(-(bd * bd) / var)
    ex = np.concatenate([de, bf], axis=1)

    def mlp(x):
        h = np.maximum(x @ W1.T + b1, 0.0)
        return np.maximum(h @ W2.T + b2, 0.0)

    er = mlp(ee.reshape(B, C * E))                # (B, H2)
    cr = mlp(ce.reshape(B * N, C * E)).reshape(B, N, H2)
    ern = er / np.maximum(np.linalg.norm(er, axis=-1, keepdims=True), 1e-12)
    crn = cr / np.maximum(np.linalg.norm(cr, axis=-1, keepdims=True), 1e-12)
    tr = np.einsum("bd,bnd->bn", ern, crn)        # (B, N)
    mus = np.array(MUS, np.float32)
    sig = np.array(SIGMAS, np.float32)
    kk = np.exp(-((tr[..., None] - mus) ** 2) / (2.0 * sig ** 2))
    pooled = kk.sum(axis=1)
    kp = np.log(np.clip(pooled, 1e-10, None)) * 0.01
    af = np.concatenate([ex, kp], axis=1)
    sc = af @ Wc[0] + bc[0]
    return (1.0 / (1.0 + np.exp(-sc)))[:, None].astype(np.float32)


def kernel(**inputs) -> np.ndarray:
    ctx = np.asarray(inputs["batch_context"], np.int64)
    ev = np.asarray(inputs["batch_event"], np.int64)
    # BC*N + BC = 8256 quad keys per shard always fit the 16K-row table;
    # the fallback only guards pathological inputs (e.g. out-of-range ids)
    if ctx.min() < 0 or ctx.max() > V or ev.min() < 0 or ev.max() > V:
        return _numpy_reference(inputs)
    nc = _build_program(True)
    table8 = (np.asarray(inputs["event_table"], np.float32) * 8.0).astype(FP8NP)
    in_maps = [
        _prep_core_inputs(inputs, core, True, table8) for core in range(NCORES)
    ]
    res = run_bass_kernel_spmd(nc, in_maps, core_ids=list(range(NCORES)))
    return np.concatenate([r["out"] for r in res.results], axis=0)


if __name__ == "__main__":
    nc = _build_program(True)
    print("program built ok")


# revision 81
# speedup vs baseline: 1.1726x; 1.1726x over previous
"""Trainium2 Bass kernel for nn_EventPairCompositionModel (fp8 DoubleRow).

Strategy (data-parallel over batch, 8 cores, B=512 -> 64 per core):
  - Host builds a per-core compact table of QUAD rows: the ~8.2K unique
    (idx0..idx3) context/event tuples each shard touches, stored as fp8e4m3
    (x8 scale) 1200-elem rows padded to 1280 bytes and DECLARED int16 so the
    SWDGE transpose-gather's 16-bit granularity lands fp8 element pairs
    (2u, 2u+1) on partition u%128 -- exactly the [K, 2, N] layout
    MatmulPerfMode.DoubleRow wants.  Quad rows give the minimal 5 DoubleRow
    K-pairs per m-tile and 1 gather index per (b, n) (the SWDGE ucode is
    per-index bound; bursts over ~512 idx deadlock its descriptor ring).
  - Context MLP (1200->512->256) entirely in fp8 DoubleRow (2 K-rows/cycle).
    Weight passes of 3 groups of 512 (b,n) pairs amortize each stationary
    load over 3 matmuls while leaving PSUM for a 3-deep MLP2 ring (no
    act-wait bubble at pass boundaries); redundant LDWEIGHTS are removed by
    a post-compile pass.  The 64 event tuples ride pass 0 as a narrow group.
  - Cosine numerators/|c|^2 via per-batch fp8 DR matmuls ([p,2,1]
    stationaries) deferred into the NEXT pass's matmul stream, staged to
    [1, 8192] rows and reshaped to [64 batch, 128 ctx] by one DMA; batched
    KNRM pooling in two column chunks x two batch halves (first half
    mid-stream).  All scalar activations live in one function table
    (sqrt/sigmoid rewritten via exp/ln) so no mid-kernel table reloads.
  - Distance-kernel path in bf16 off the event gather; final linear+sigmoid
    in [64 batch partitions, feature] layout.
All 8 cores run the identical program on their own batch shard (SPMD).
Host falls back to a numpy reference for pathological inputs (out-of-range
indices); the quad table always fits for well-formed ones.
"""

import numpy as np
import ml_dtypes

import concourse.bacc as bacc
import concourse.bass as bass
import concourse.tile as tile
import concourse.mybir as mybir
from concourse.bass_utils import run_bass_kernel_spmd
from concourse import library_config

F32 = mybir.dt.float32
BF16 = mybir.dt.bfloat16
FP8 = mybir.dt.float8e4
I16 = mybir.dt.int16
AF = mybir.ActivationFunctionType
DR = mybir.MatmulPerfMode.DoubleRow

# Problem shapes (hardcoded per spec)
B, N, C, E = 512, 128, 4, 300
V = 50000
H1, H2 = 512, 256
NF, NK = 8, 11
NCORES = 8
BC = B // NCORES          # 64 batches per core
CT = 16384                # compact quad-table rows (int16-indexable)
EP = 1200                 # elems per quad row (all 4 components)
RU = 640                  # int16 units per table row (1280 fp8 elems)
GROUPS = (BC * N) // 512  # 16 groups of 512 (b,n) pairs
NKP = 5                   # MLP1 K-pairs per m-tile (5 slots of 256 elems)
FD = 28                   # features: 9 dist + 8 extracted + 11 kp
# weight passes: 3 groups share each stationary load; PSUM then affords a
# 3-deep MLP2 ring (no act-wait bubble at pass boundaries)
PASS_GROUPS = [[0, 1, 2], [3, 4, 5], [6, 7, 8], [9, 10, 11], [12, 13], [14, 15]]

MUS = [1.0, 0.9, 0.7, 0.5, 0.3, 0.1, -0.1, -0.3, -0.5, -0.7, -0.9]
SIGMAS = [1e-3] + [0.1] * 10

_PROGRAM_CACHE = {}


def _build_program(fast: bool = True):
    if True in _PROGRAM_CACHE:
        return _PROGRAM_CACHE[True]

    # Force the activation-table placement pass to use the combined
    # natural_log_exp_and_others set (covers relu/exp/ln/square/copy): blank
    # the narrower sets, keeping list positions so act_func_set_ids stay
    # aligned with act_info.json.  Every function this kernel uses then lives
    # in one table -> no 1.3us ACT_TABLE_LOADs between activations.
    from concourse import hw_specs as _hw
    if not hasattr(bacc, "_orig_gat"):
        bacc._orig_gat = bacc.get_activation_tables

        def _gat(arch):
            t = dict(bacc._orig_gat(arch))
            keys = list(t.keys())
            for i, k in enumerate(keys):
                if k != "natural_log_exp_and_others" and i < 6:
                    t[k] = set()
            return t

        bacc.get_activation_tables = _gat

    nc = bacc.Bacc("TRN2", target_bir_lowering=False, debug=False, num_swdge_queues=4)

    # ---- DRAM I/O ----
    ctab = nc.dram_tensor("ctab", (CT, RU), I16, kind="ExternalInput")
    cidx = nc.dram_tensor("cidx", (128, GROUPS * 2 * 16), I16, kind="ExternalInput")
    eidx = nc.dram_tensor("eidx", (128, 8), I16, kind="ExternalInput")
    w1p = nc.dram_tensor("w1p", (128, NKP * 4 * 256), FP8, kind="ExternalInput")
    w2p = nc.dram_tensor("w2p", (128, 2 * 2 * 256), FP8, kind="ExternalInput")
    wvp = nc.dram_tensor("wvp", (128, 4 * 9), BF16, kind="ExternalInput")
    b1d = nc.dram_tensor("b1d", (128, 4), F32, kind="ExternalInput")
    b1f = nc.dram_tensor("b1f", (128, 4), F32, kind="ExternalInput")
    b2d = nc.dram_tensor("b2d", (128, 2), F32, kind="ExternalInput")
    ebv = nc.dram_tensor("ebv", (BC, 9), F32, kind="ExternalInput")
    ndsq = nc.dram_tensor("ndsq", (BC, 9), F32, kind="ExternalInput")
    featd = nc.dram_tensor("featd", (BC, NF), F32, kind="ExternalInput")
    wcr = nc.dram_tensor("wcr", (BC, FD), F32, kind="ExternalInput")
    bcr = nc.dram_tensor("bcr", (BC, 1), F32, kind="ExternalInput")
    out_d = nc.dram_tensor("out", (BC, 1), F32, kind="ExternalOutput")

    with tile.TileContext(nc) as tc:
        with (
            tc.tile_pool(name="consts", bufs=1) as cpool,
            tc.tile_pool(name="xt", bufs=9) as xtpool,
            tc.tile_pool(name="s1", bufs=6) as s1pool,
            tc.tile_pool(name="s2", bufs=8) as s2pool,
            tc.tile_pool(name="csq", bufs=8) as csqpool,
            tc.tile_pool(name="small", bufs=2) as smpool,
            tc.tile_pool(name="pm1", bufs=3, space="PSUM") as pm1,
            tc.tile_pool(name="pm2", bufs=3, space="PSUM") as pm2,
            tc.tile_pool(name="pg", bufs=2, space="PSUM") as pgpool,
        ):
            nc.gpsimd.load_library(library_config.mlp)

            # ---- load constants ----
            cidx_s = cpool.tile([128, GROUPS * 2 * 16], I16)
            nc.sync.dma_start(cidx_s[:], cidx.ap())
            eidx_s = cpool.tile([128, 8], I16)
            nc.sync.dma_start(eidx_s[:], eidx.ap())
            w1p_s = cpool.tile([128, NKP * 4 * 256], FP8)
            nc.sync.dma_start(w1p_s[:], w1p.ap())
            w2p_s = cpool.tile([128, 2 * 2 * 256], FP8)
            nc.scalar.dma_start(w2p_s[:], w2p.ap())
            wvp_s = cpool.tile([128, 4 * 9], BF16)
            nc.scalar.dma_start(wvp_s[:], wvp.ap())
            b1_s = cpool.tile([128, 4], F32)
            nc.sync.dma_start(b1_s[:], b1d.ap())
            b1f_s = cpool.tile([128, 4], F32)
            nc.sync.dma_start(b1f_s[:], b1f.ap())
            b2_s = cpool.tile([128, 2], F32)
            nc.sync.dma_start(b2_s[:], b2d.ap())
            ebv_s = cpool.tile([BC, 9], F32)
            nc.scalar.dma_start(ebv_s[:], ebv.ap())
            ndsq_s = cpool.tile([BC, 9], F32)
            nc.scalar.dma_start(ndsq_s[:], ndsq.ap())
            wcr_s = cpool.tile([BC, FD], F32)
            nc.scalar.dma_start(wcr_s[:], wcr.ap())
            bcr_s = cpool.tile([BC, 1], F32)
            nc.scalar.dma_start(bcr_s[:], bcr.ap())
            F_s = cpool.tile([BC, FD], F32)
            nc.sync.dma_start(F_s[:, 9 : 9 + NF], featd.ap())

            # ones pair for |c|^2 matmuls; K-group dim at stride 16 to satisfy
            # the dual-fp8 LDWEIGHTS AP restriction (pair step % 16 == 0)
            ones2c_s = cpool.tile([128, 32], FP8)
            nc.vector.memset(ones2c_s[:], 1.0)
            # batched KNRM constants: [64, k(11), 128] of -mu_k / -1/(2 sig_k^2)
            mub_s = cpool.tile([BC, NK * 128], F32)
            i2s_s = cpool.tile([BC, NK * 128], F32)
            for k in range(NK):
                nc.vector.memset(mub_s[:, 128 * k : 128 * (k + 1)], -MUS[k])
                nc.vector.memset(
                    i2s_s[:, 128 * k : 128 * (k + 1)],
                    -1.0 / (2.0 * SIGMAS[k] ** 2),
                )

            # persistent accumulators
            traw_s = cpool.tile([BC, 128], F32)        # 16*dot per (b, n)
            drow_s = cpool.tile([1, 512 * GROUPS], F32)  # dots, (g,s,n) on part 0
            nrow_s = cpool.tile([1, 512 * GROUPS], F32)  # |c|^2 likewise
            sgram_s = cpool.tile([128, 2 * BC], FP8)   # event reprs [p, m(2), 64]
            cse_s = cpool.tile([128, 2 * BC], FP8)     # their squares
            s1e_s = cpool.tile([128, 4 * BC], FP8)     # event s1 [p, mj(4), 64]
            predb_s = cpool.tile([128, 4 * BC], BF16)  # predicates [p, f(4), 64]
            ne2_s = cpool.tile([BC, 1], F32)
            ncsq0_s = cpool.tile([BC, 128], F32)
            trans_s = cpool.tile([BC, 128], F32)
            pooled_s = cpool.tile([BC, NK], F32)

            # ---- gathers (issued lazily so pool-slot reuse stays WAR-safe) ----
            xe_s = cpool.tile([128, 5 * 128], I16)   # event quad gather
            xts = {}

            def issue_gather(g):
                # two 256-idx gathers per group (small bursts keep the SWDGE
                # descriptor-ring carveout happy; big ones deadlock it)
                if g >= GROUPS:
                    return
                xt = xtpool.tile([128, 5 * 512], I16, tag="xt", name=f"xt{g}")
                nc.gpsimd.dma_gather(
                    out_ap=xt[:].rearrange("p (s r) -> p s r", s=5),
                    in_ap=ctab.ap(),
                    idxs_ap=cidx_s[:, 32 * g : 32 * (g + 1)],
                    num_idxs=512,
                    num_idxs_reg=512,
                    elem_size=RU,
                    transpose=True,
                )
                xts[g] = xt

            # g0 first (it gates the first matmul); events slot in behind it
            issue_gather(0)
            nc.gpsimd.dma_gather(
                out_ap=xe_s[:].rearrange("p (s i) -> p s i", s=5),
                in_ap=ctab.ap(),
                idxs_ap=eidx_s[:],
                num_idxs=128,
                num_idxs_reg=128,
                elem_size=RU,
                transpose=True,
            )
            for g in PASS_GROUPS[0][1:] + PASS_GROUPS[1]:
                issue_gather(g)

            def ctx_rhs(g, uj, w):
                # [p, 2(byte), w cols] fp8 view: group g, K-slot uj
                v = xts[g][:].bitcast(FP8).rearrange(
                    "p (s r i) -> p s i r", s=5, i=2
                )
                return v[:, uj, :, 0:w]

            def evt_rhs(uj):
                v = xe_s[:].bitcast(FP8).rearrange("p (s r i) -> p s i r", s=5, i=2)
                return v[:, uj, :, 0:BC]

            def w1_ap(kp, m):
                return w1p_s[:].rearrange(
                    "p (kp m i c) -> p kp m i c", kp=NKP, m=4, i=2
                )[:, kp, m, :, :]

            def w2_ap(q, m):
                return w2p_s[:].rearrange(
                    "p (q m i c) -> p q m i c", q=2, m=2, i=2
                )[:, q, m, :, :]

            # deferred per-batch dot/|c|^2 matmuls: flushed into the NEXT
            # pass's matmul stream so their dependency chains (act -> square)
            # never stall the PE at pass boundaries.  Outputs pack 4 streams
            # per PSUM bank at 32-aligned partitions.
            pending = []

            flush_n = [0]

            def flush_step(nstreams=2):
                # emit up to nstreams deferred dot/norm streams; called once
                # per m-slot of the following pass so the ring-2 PSUM recycle
                # always has a full m-window of slack
                sg_v = sgram_s[:].rearrange("p (m c) -> p m c", m=2)
                on_v = ones2c_s[:].rearrange("p (i x) -> p i x", i=2)[:, :, 0:1]
                for _ in range(nstreams):
                    if not pending:
                        return
                    kind, g, mv = pending.pop(0)
                    flush_n[0] += 1
                    PD = pgpool.tile([1, 512], F32, tag="pg", name=f"pd{flush_n[0]}")
                    mv_v = mv[:].rearrange("p (m x) -> p m x", m=2)
                    for s in range(4):
                        lane = 4 * g + s
                        nc.tensor.matmul(
                            PD[:, 128 * s : 128 * (s + 1)],
                            sg_v[:, :, lane : lane + 1] if kind == "d" else on_v,
                            mv_v[:, :, 128 * s : 128 * (s + 1)],
                            start=True, stop=True, perf_mode=DR,
                        )
                    if kind == "d":
                        nc.scalar.copy(drow_s[:, 512 * g : 512 * (g + 1)], PD[:])
                    else:
                        nc.vector.tensor_copy(
                            out=nrow_s[:, 512 * g : 512 * (g + 1)], in_=PD[:]
                        )

            def tail_half(h):
                # cosine + batched KNRM pooling for batches 32h..32h+31
                r = slice(32 * h, 32 * (h + 1))
                # scalar-issued: the sync queue can lag behind on semaphore
                # bookkeeping right when these need to fire
                nc.scalar.dma_start(
                    traw_s[r, :], drow_s[:, 4096 * h : 4096 * (h + 1)]
                )
                nc.scalar.dma_start(
                    ncsq0_s[r, :], nrow_s[:, 4096 * h : 4096 * (h + 1)]
                )
                prodn = smpool.tile([BC, 128], F32, tag="smT", name=f"prodn{h}")
                nc.vector.tensor_tensor(
                    out=prodn[r, :], in0=ncsq0_s[r, :],
                    in1=ne2_s[r, :].broadcast_to([32, 128]),
                    op=mybir.AluOpType.mult,
                )
                # 1/sqrt(x) = exp(-0.5 ln x): stays inside the ln+exp
                # activation-table set (table reloads cost 1.3us each)
                lnp = smpool.tile([BC, 128], F32, tag="smT", name=f"lnp{h}")
                nc.scalar.activation(lnp[r, :], prodn[r, :], AF.Ln)
                nf = smpool.tile([BC, 128], F32, tag="smT", name=f"nf{h}")
                nc.scalar.activation(nf[r, :], lnp[r, :], AF.Exp, scale=-0.5)
                nc.vector.tensor_mul(trans_s[r, :], traw_s[r, :], nf[r, :])

                # two kernel-chunks so the scalar Exp of chunk A overlaps the
                # DVE chain of chunk B
                ekbs = []
                for k0, k1 in ((0, 6), (6, NK)):
                    c = slice(128 * k0, 128 * k1)
                    nk = k1 - k0
                    dk = smpool.tile(
                        [BC, NK * 128], F32, tag="smB", name=f"dk{h}{k0}"
                    )
                    nc.vector.tensor_tensor(
                        out=dk[r, c],
                        in0=trans_s[r, :][:, None, :].broadcast_to([32, nk, 128]),
                        in1=mub_s[r, c].rearrange("b (k n) -> b k n", k=nk),
                        op=mybir.AluOpType.add,
                    )
                    dsq = smpool.tile(
                        [BC, NK * 128], F32, tag="smB", name=f"dsq{h}{k0}"
                    )
                    nc.vector.tensor_mul(dsq[r, c], dk[r, c], dk[r, c])
                    argb = smpool.tile(
                        [BC, NK * 128], F32, tag="smB", name=f"argb{h}{k0}"
                    )
                    nc.vector.tensor_mul(argb[r, c], dsq[r, c], i2s_s[r, c])
                    argc = smpool.tile(
                        [BC, NK * 128], F32, tag="smB", name=f"argc{h}{k0}"
                    )
                    nc.vector.tensor_scalar_max(argc[r, c], argb[r, c], -87.0)
                    ekb = smpool.tile(
                        [BC, NK * 128], F32, tag=f"smE{k0}", name=f"ekb{h}{k0}"
                    )
                    nc.scalar.activation(ekb[r, c], argc[r, c], AF.Exp)
                    ekbs.append((k0, k1, ekb))
                for k0, k1, ekb in ekbs:
                    c = slice(128 * k0, 128 * k1)
                    nc.vector.reduce_sum(
                        out=pooled_s[r, k0:k1],
                        in_=ekb[r, c].rearrange("b (k n) -> b k n", k=k1 - k0),
                        axis=mybir.AxisListType.X,
                    )

            # ---- weight passes (+ events on pass 0) ----
            for pi, grp in enumerate(PASS_GROUPS):
                with_evt = pi == 0
                # MLP1
                s1t = {}
                for g in grp:
                    s1t[g] = s1pool.tile([128, 4 * 512], FP8, tag="s1", name=f"s1_{g}")
                pl = {}
                for m in range(4):
                    for g in grp:
                        pl[g] = pm1.tile([128, 512], F32, tag="pm1", name=f"p1_{g}_{m}")
                    if with_evt:
                        pE = pm2.tile([128, BC], F32, tag="pm2", name=f"pe_{m}")
                    for kp in range(NKP):
                        st, sp = kp == 0, kp == NKP - 1
                        for g in grp:
                            nc.tensor.matmul(
                                pl[g][:], w1_ap(kp, m), ctx_rhs(g, kp, 512),
                                start=st, stop=sp, perf_mode=DR,
                            )
                        if with_evt:
                            nc.tensor.matmul(
                                pE[:], w1_ap(kp, m), evt_rhs(kp),
                                start=st, stop=sp, perf_mode=DR,
                            )
                    # acts first (PSUM recycle gates the next m-slot's
                    # matmuls), then the deferred dots ride behind them
                    for g in grp:
                        nc.scalar.activation(
                            s1t[g][:, 512 * m : 512 * (m + 1)], pl[g][:],
                            AF.Relu, bias=b1_s[:, m : m + 1], scale=1.0 / 16,
                        )
                    flush_step(2)
                    if with_evt:
                        nc.scalar.activation(
                            s1e_s[:, BC * m : BC * (m + 1)], pE[:],
                            AF.Relu, bias=b1_s[:, m : m + 1], scale=1.0 / 16,
                        )

                # MLP2 in half-passes of <=2 groups (ring-3 PSUM slack)
                s2t = {}
                for g in grp:
                    s2t[g] = s2pool.tile([128, 2 * 512], FP8, tag="s2", name=f"s2_{g}")
                for half in range((len(grp) + 1) // 2):
                    gh = grp[2 * half : 2 * half + 2]
                    for m in range(2):
                        p2 = {}
                        for g in gh:
                            p2[g] = pm2.tile(
                                [128, 512], F32, tag="pm2", name=f"p2_{g}_{m}"
                            )
                        if with_evt and half == 0:
                            pE2 = pm2.tile([128, BC], F32, tag="pm2", name=f"pe2_{m}")
                        for q in range(2):
                            st, sp = q == 0, q == 1
                            for g in gh:
                                nc.tensor.matmul(
                                    p2[g][:], w2_ap(q, m),
                                    s1t[g][:].rearrange("p (mj x) -> p mj x", mj=4)[
                                        :, 2 * q : 2 * q + 2, :
                                    ],
                                    start=st, stop=sp, perf_mode=DR,
                                )
                            if with_evt and half == 0:
                                nc.tensor.matmul(
                                    pE2[:], w2_ap(q, m),
                                    s1e_s[:].rearrange("p (mj x) -> p mj x", mj=4)[
                                        :, 2 * q : 2 * q + 2, :
                                    ],
                                    start=st, stop=sp, perf_mode=DR,
                                )
                        for g in gh:
                            nc.scalar.activation(
                                s2t[g][:, 512 * m : 512 * (m + 1)], p2[g][:],
                                AF.Relu, bias=b2_s[:, m : m + 1], scale=1.0 / 8,
                            )
                        if with_evt and half == 0:
                            nc.scalar.activation(
                                sgram_s[:, BC * m : BC * (m + 1)], pE2[:],
                                AF.Relu, bias=b2_s[:, m : m + 1], scale=1.0 / 8,
                            )

                    if with_evt and half == 0:
                        # event extras: squares, |e|^2, predicates, variances
                        nc.vector.tensor_mul(cse_s[:], sgram_s[:], sgram_s[:])
                        pne = pm2.tile([BC, 1], F32, tag="pm2", name="pne")
                        nc.tensor.matmul(
                            pne[:],
                            cse_s[:].rearrange("p (m c) -> p m c", m=2),
                            ones2c_s[:, 0:2].rearrange("p (o i) -> p i o", i=2),
                            start=True, stop=True, perf_mode=DR,
                        )
                        nc.scalar.copy(ne2_s[:], pne[:])

                        # predicates: quad-row elems 300..599 (component 1)
                        nc.scalar.copy(
                            predb_s[:].rearrange("p (s i l) -> p s i l", s=2, i=2),
                            xe_s[:].bitcast(FP8).rearrange(
                                "p (s r i) -> p s i r", s=5, i=2
                            )[:, 1:3, :, 0:BC],
                        )
                        pvar = pm2.tile([BC, 9], F32, tag="pm2", name="pvar")
                        for f in range(4):
                            nc.tensor.matmul(
                                pvar[:],
                                predb_s[:].rearrange("p (f l) -> p f l", f=4)[:, f, :],
                                wvp_s[:].rearrange("p (f v) -> p f v", f=4)[:, f, :],
                                start=(f == 0), stop=(f == 3),
                            )
                        ez = smpool.tile([BC, 9], F32, tag="sm9", name="ez")
                        nc.scalar.activation(ez[:], pvar[:], AF.Exp, scale=1.0 / 8)
                        ezb = smpool.tile([BC, 9], F32, tag="sm9", name="ezb")
                        nc.vector.tensor_mul(ezb[:], ez[:], ebv_s[:])
                        ez1 = smpool.tile([BC, 9], F32, tag="sm9", name="ez1")
                        nc.vector.tensor_scalar_add(ez1[:], ezb[:], 1.0)
                        var = smpool.tile([BC, 9], F32, tag="sm9", name="var")
                        nc.scalar.activation(var[:], ez1[:], AF.Ln)
                        rv = smpool.tile([BC, 9], F32, tag="sm9", name="rv")
                        nc.vector.reciprocal(rv[:], var[:])
                        qd = smpool.tile([BC, 9], F32, tag="sm9", name="qd")
                        nc.vector.tensor_mul(qd[:], ndsq_s[:], rv[:])
                        nc.scalar.activation(F_s[:, 0:9], qd[:], AF.Exp)

                    # squared activations now; dot/norm matmuls deferred
                    for g in gh:
                        csq = csqpool.tile(
                            [128, 2 * 512], FP8, tag="csq", name=f"csq_{g}"
                        )
                        nc.vector.tensor_mul(csq[:], s2t[g][:], s2t[g][:])
                        pending.append(("d", g, s2t[g]))
                        pending.append(("n", g, csq))
                # prefetch the gathers needed two passes ahead
                if pi + 2 < len(PASS_GROUPS):
                    for g in PASS_GROUPS[pi + 2]:
                        issue_gather(g)
                if pi == 3:
                    tail_half(0)   # batches 0..31 finished flushing by now
            while pending:
                flush_step(2)
            tail_half(1)

            # ---- final score ----
            poolc = smpool.tile([BC, NK], F32, tag="smK", name="poolc")
            nc.vector.tensor_scalar_max(poolc[:], pooled_s[:], 1e-10)
            nc.scalar.activation(F_s[:, 9 + NF :], poolc[:], AF.Ln)

            fw = smpool.tile([BC, FD], F32, tag="smK", name="fw")
            nc.vector.tensor_mul(fw[:], F_s[:], wcr_s[:])
            sc = smpool.tile([BC, 1], F32, tag="smS", name="sc")
            nc.vector.reduce_sum(out=sc[:], in_=fw[:], axis=mybir.AxisListType.X)
            # sigmoid via exp (avoids a sigmoid-table load): 1/(1+e^(-x-bc))
            emx = smpool.tile([BC, 1], F32, tag="smS", name="emx")
            nc.scalar.activation(emx[:], sc[:], AF.Exp, bias=bcr_s[:], scale=-1.0)
            em1 = smpool.tile([BC, 1], F32, tag="smS", name="em1")
            nc.vector.tensor_scalar_add(em1[:], emx[:], 1.0)
            sig = smpool.tile([BC, 1], F32, tag="smS", name="sig")
            nc.vector.reciprocal(sig[:], em1[:])
            nc.scalar.dma_start(out_d.ap(), sig[:])

    nc.compile()

    # Spread SWDGE gathers across the 4 queues (ucode locks each DMASW
    # semaphore lane to one queue; lanes are assigned round-robin in
    # scheduled order).
    import re as _re
    for blk in nc.m.functions[0].blocks:
        for inst in blk.instructions:
            if type(inst).__name__ == "InstDMAGatherAnt":
                for u in inst.sync_info.on_update:
                    m = _re.match(r"DMASW(\d+)_", u.ant_name or "")
                    if m:
                        inst.queue_num = int(m.group(1)) % 4
                        break

    _dedup_ldweights(nc)

    _PROGRAM_CACHE[True] = nc
    return nc


def _ldw_sig(inst):
    a = inst.ins[0]
    return (
        a.memref,
        a.offset,
        tuple(tuple(d) for d in a.ap),
        getattr(inst, "perf_mode", None),
        getattr(inst, "tile_position", None),
        getattr(inst, "tile_size", None),
        getattr(inst, "is_transpose", None),
    )


def _dedup_ldweights(nc):
    """Remove InstLdweights that reload the stationary operand already in the
    PE array.  The compile pass splits every matmul into LDWEIGHTS+MATMUL;
    back-to-back matmuls sharing weights then pay a redundant ~200ns load.
    Conservative: only drops loads carrying no semaphore waits/updates, so
    cross-engine ordering is untouched."""
    dropped = 0
    for blk in nc.m.functions[0].blocks:
        cur = None          # signature currently in the array
        keep = []
        for inst in blk.instructions:
            nm = type(inst).__name__
            if nm == "InstLdweights":
                sig = _ldw_sig(inst)
                si = inst.sync_info
                if sig == cur and (
                    si is None or (not si.on_wait and not si.on_update)
                ):
                    dropped += 1
                    continue
                cur = sig
            keep.append(inst)
        blk.instructions = keep
    return dropped


def _wrap16(flat_idx):
    """int16 index list -> (128, n/16) tile layout replicated into 8 stripes."""
    n = flat_idx.shape[0]
    t = np.zeros((16, n // 16), np.int16)
    t[np.arange(n) % 16, np.arange(n) // 16] = flat_idx
    return np.tile(t, (8, 1))


FP8NP = ml_dtypes.float8_e4m3fn


def _prep_core_inputs(inputs, core, fast=True, table8=None):
    """Host-side shard + weight re-layouts for one core."""
    W1 = np.asarray(inputs["W1"], np.float32)
    W2 = np.asarray(inputs["W2"], np.float32)
    Wv = np.asarray(inputs["Wv"], np.float32)
    Wc = np.asarray(inputs["Wc"], np.float32)
    b1 = np.asarray(inputs["b1"], np.float32)
    b2 = np.asarray(inputs["b2"], np.float32)
    bv = np.asarray(inputs["bv"], np.float32)
    bc = np.asarray(inputs["bc"], np.float32)

    sl = slice(core * BC, (core + 1) * BC)
    ev = np.asarray(inputs["batch_event"][sl], np.int64)          # (BC, C)
    feats = np.asarray(inputs["batch_features"][sl], np.float32)  # (BC, NF)
    dists = np.asarray(inputs["batch_distances"][sl], np.float32) # (BC, 9)
    ctx = np.asarray(inputs["batch_context"][sl], np.int64)       # (BC, N, C)

    if table8 is None:
        table8 = (np.asarray(inputs["event_table"], np.float32) * 8.0).astype(FP8NP)

    # quad keys: the full (idx0..idx3) tuple per (b, n) / event
    Vp = np.int64(V + 1)
    ctxq = ctx.reshape(BC * N, 4)
    evq = ev.reshape(BC, 4)

    def qkey(a):
        return ((a[:, 0] * Vp + a[:, 1]) * Vp + a[:, 2]) * Vp + a[:, 3]

    keys = np.concatenate([qkey(ctxq), qkey(evq)])
    uniq, inv = np.unique(keys, return_inverse=True)
    assert len(uniq) <= CT
    ctab8 = np.zeros((CT, 2 * RU), FP8NP)
    rem = uniq.copy()
    for c in range(3, -1, -1):
        ctab8[: len(uniq), E * c : E * (c + 1)] = table8[rem % Vp]
        rem //= Vp
    rctx = inv[: BC * N].astype(np.int16).reshape(BC, N)
    rev = inv[BC * N :].astype(np.int16)

    # context gathers: per group g, 512 idxs ordered (s, n)
    ci = rctx.reshape(GROUPS, 512)
    cidx = np.concatenate(
        [_wrap16(ci[g]) for g in range(GROUPS)], axis=1
    )
    # event gather: 128 idxs; lanes >= BC gather row 0
    ei = np.zeros(128, np.int16)
    ei[:BC] = rev

    # W1 packed for DoubleRow: [p, kp(uj), m, i, mcol]
    W1x = (8.0 * W1).astype(np.float32)          # (H1, C*E)
    W2x = (8.0 * W2).astype(np.float32)          # (H2, H1)
    p_i = np.arange(128)
    w1p = np.zeros((128, NKP, 4, 2, 128), np.float32)
    for uj in range(NKP):
        e = 256 * uj + 2 * p_i[:, None] + np.arange(2)[None, :]  # (128, 2)
        valid = e < EP
        src = W1x[:, np.minimum(e, EP - 1)] * valid[None, :, :]   # (H1, 128, 2)
        blk = src.reshape(4, 128, 128, 2).transpose(2, 0, 3, 1)
        w1p[:, uj] = blk
    w2p = np.zeros((128, 2, 2, 2, 128), np.float32)
    for q in range(2):
        for i in range(2):
            src = W2x[:, 128 * (2 * q + i) + p_i]      # (H2, 128)
            w2p[:, q, :, i, :] = src.reshape(2, 128, 128).transpose(2, 0, 1)
    # predicates live at quad-row elems 300..599 (component 1):
    # f slots are (uj, i) for uj in {1, 2}
    wvp = np.zeros((128, 4, 9), np.float32)
    for f in range(4):
        e = 256 * (1 + f // 2) + 2 * p_i + (f % 2)
        k = e - E
        valid = (k >= 0) & (k < E)
        wvp[:, f, :] = Wv[:, np.clip(k, 0, E - 1)].T * valid[:, None]

    wc_r = np.concatenate(
        [Wc[0, 0:9], Wc[0, 9 : 9 + NF], Wc[0, 9 + NF :] * 0.01]
    ).astype(np.float32)

    m = {
        "ctab": np.ascontiguousarray(ctab8).view(np.int16),
        "cidx": np.ascontiguousarray(cidx),
        "eidx": np.ascontiguousarray(_wrap16(ei)),
        "w1p": w1p.reshape(128, -1).astype(FP8NP),
        "w2p": w2p.reshape(128, -1).astype(FP8NP),
        "wvp": wvp.reshape(128, -1).astype(ml_dtypes.bfloat16),
        "b1d": np.ascontiguousarray(4.0 * b1.reshape(4, 128).T),
        "b1f": np.ascontiguousarray(64.0 * b1.reshape(4, 128).T),
        "b2d": np.ascontiguousarray(4.0 * b2.reshape(2, 128).T),
        "ebv": np.tile(np.exp(bv)[None, :], (BC, 1)).astype(np.float32),
        "ndsq": np.ascontiguousarray(-(dists * dists)),
        "featd": np.ascontiguousarray(feats),
        "wcr": np.tile(wc_r[None, :], (BC, 1)),
        "bcr": np.full((BC, 1), -bc[0], np.float32),
    }
    return m


def _numpy_reference(inputs):
    """Pure-host fallback (unreachable for the spec's random fill)."""
    t = np.asarray(inputs["event_table"], np.float32)
    W1 = np.asarray(inputs["W1"], np.float32); b1 = np.asarray(inputs["b1"], np.float32)
    W2 = np.asarray(inputs["W2"], np.float32); b2 = np.asarray(inputs["b2"], np.float32)
    Wv = np.asarray(inputs["Wv"], np.float32); bv = np.asarray(inputs["bv"], np.float32)
    Wc = np.asarray(inputs["Wc"], np.float32); bc = np.asarray(inputs["bc"], np.float32)
    be = np.asarray(inputs["batch_event"], np.int64)
    bf = np.asarray(inputs["batch_features"], np.float32)
    bd = np.asarray(inputs["batch_distances"], np.float32)
    bx = np.asarray(inputs["batch_context"], np.int64)
    ee = t[be]                                    # (B, C, E)
    ce = t[bx]                                    # (B, N, C, E)
    pred = ee[:, 1, :]
    zv = pred @ Wv.T + bv
    var = np.log1p(np.exp(zv))
    de = np.exp(-(bd * bd) / var)
    ex = np.concatenate([de, bf], axis=1)

    def mlp(x):
        h = np.maximum(x @ W1.T + b1, 0.0)
        return np.maximum(h @ W2.T + b2, 0.0)

    er = mlp(ee.reshape(B, C * E))                # (B, H2)
    cr = mlp(ce.reshape(B * N, C * E)).reshape(B, N, H2)
    ern = er / np.maximum(np.linalg.norm(er, axis=-1, keepdims=True), 1e-12)
    crn = cr / np.maximum(np.linalg.norm(cr, axis=-1, keepdims=True), 1e-12)
    tr = np.einsum("bd,bnd->bn", ern, crn)        # (B, N)
    mus = np.array(MUS, np.float32)
    sig = np.array(SIGMAS, np.float32)
    kk = np.exp(-((tr[..., None] - mus) ** 2) / (2.0 * sig ** 2))
    pooled = kk.sum(axis=1)
    kp = np.log(np.clip(pooled, 1e-10, None)) * 0.01
    af = np.concatenate([ex, kp], axis=1)
    sc = af @ Wc[0] + bc[0]
    return (1.0 / (1.0 + np.exp(-sc)))[:, None].astype(np.float32)


def kernel(**inputs) -> np.ndarray:
    ctx = np.asarray(inputs["batch_context"], np.int64)
    ev = np.asarray(inputs["batch_event"], np.int64)
    # BC*N + BC = 8256 quad keys per shard always fit the 16K-row table;
    # the fallback only guards pathological inputs (e.g. out-of-range ids)
    if ctx.min() < 0 or ctx.max() > V or ev.min() < 0 or ev.max() > V:
        return _numpy_reference(inputs)
    nc = _build_program(True)
    table8 = (np.asarray(inputs["event_table"], np.float32) * 8.0).astype(FP8NP)
    in_maps = [
        _prep_core_inputs(inputs, core, True, table8) for core in range(NCORES)
    ]
    res = run_bass_kernel_spmd(nc, in_maps, core_ids=list(range(NCORES)))
    return np.concatenate([r["out"] for r in res.results], axis=0)


if __name__ == "__main__":
    nc = _build_program(True)
    print("program built ok")
